# revision 18
# baseline (speedup 1.0000x reference)
"""Trainium2 Bass kernel for nn_ColorResBlock (LayerNorm + color-structured conv + ReLU residual).

Math (reference):
    h  = LayerNorm(x) over last axis (3072), affine (gamma,beta) tied per k across 6 colors
    xr = h.reshape(B, 24, 512, 6)
    out[b,i,o,d] = sum_k alpha[o,k] xr[b,i,k,d] + sum_k beta[o,k] xsum[b,i,k] + bias[o]
    result = x + relu(out)

Key algebraic restructuring (all folds computed on host):
    A2[o,k]  = (W0-W1)[o,k] * gamma[k]
    C1[o]    = -(sum_k (W0-W1)[o,k] g[k]) - 6*sum_k W1[o,k] g[k]
    B3[o,k]  = W1[o,k]*g[k] + C1[o]/3072          (folds the mean*C1 rank-1 term)
    C0[o]    = (W0-W1)@bn + 6*(W1@bn) + bias
    h_d[r,o] = rstd[r] * ( A2 @ x_d[r] + B3 @ xsum[r] + std[r]*C0[o] )
so the matmuls run on RAW x (de-interleaved+transposed on device via PE), and the
per-row layernorm enters only as a per-partition scale folded into the ReLU pass.

Data parallel over batch: 8 cores x 128 batches.  Per core: rows R=3072, channels 3072.
"""

import numpy as np
import concourse.bacc as bacc
import concourse.bass as bass
import concourse.tile as tile
from concourse import mybir, bass_utils

F32 = mybir.dt.float32
F32R = mybir.dt.float32r
BF16 = mybir.dt.bfloat16
AF = mybir.ActivationFunctionType
OP = mybir.AluOpType

B, NPOS, KH, NCOL = 1024, 24, 512, 6
CORES = 8
BL = B // CORES            # batches per core
R = BL * NPOS              # 3072 rows per core
C = KH * NCOL              # 3072 channels
P = 128                    # partitions per row tile
RT = R // P                # 24 row tiles
KC = KH // P               # 4 contraction chunks of 128
EPS = 1e-5

# d-plane -> engine for the PSUM->SBUF transpose copies ("dve" or "act")
COPY_ENG = ["act", "act", "act", "act", "act", "act"]

_CACHE = {}


def _build(use_c0: bool):
    nc = bacc.Bacc("TRN2", target_bir_lowering=False, debug=False)
    x_d = nc.dram_tensor("x", [R, C], F32, kind="ExternalInput").ap()
    a2_d = nc.dram_tensor("a2t", [P, KC, KH], F32, kind="ExternalInput").ap()
    b3_d = nc.dram_tensor("b3t", [P, KC, KH], F32, kind="ExternalInput").ap()
    id_d = nc.dram_tensor("ident", [P, P], F32, kind="ExternalInput").ap()
    c0_d = nc.dram_tensor("c0row", [1, KH], F32, kind="ExternalInput").ap()
    out_d = nc.dram_tensor("out", [R, C], F32, kind="ExternalOutput").ap()

    with tile.TileContext(nc) as tc:
        with tc.tile_pool(name="wgt", bufs=1) as wgt, \
             tc.tile_pool(name="big", bufs=2) as big, \
             tc.tile_pool(name="med", bufs=2) as med, \
             tc.tile_pool(name="pln", bufs=3) as pln, \
             tc.tile_pool(name="sml", bufs=2) as sml, \
             tc.tile_pool(name="pT", bufs=3, space="PSUM") as pT, \
             tc.tile_pool(name="pXS", bufs=1, space="PSUM") as pXS, \
             tc.tile_pool(name="pY", bufs=(2 if use_c0 else 3), space="PSUM") as pY, \
             tc.tile_pool(name="pZ", bufs=1, space="PSUM") as pZ:

            # ---- one-time setup: weights + identity rounded to f32r
            stage = wgt.tile([P, KC, KH], F32, tag="stage")
            nc.sync.dma_start(out=stage, in_=a2_d)
            a2t = wgt.tile([P, KC, KH], BF16)
            nc.vector.tensor_copy(out=a2t, in_=stage)
            stage2 = wgt.tile([P, KC, KH], F32, tag="stage")
            nc.sync.dma_start(out=stage2, in_=b3_d)
            b3t = wgt.tile([P, KC, KH], BF16)
            nc.vector.tensor_copy(out=b3t, in_=stage2)
            idstage = wgt.tile([P, P], F32, tag="idstage")
            nc.sync.dma_start(out=idstage, in_=id_d)
            identb = wgt.tile([P, P], BF16)
            nc.vector.tensor_copy(out=identb, in_=idstage)
            identf = idstage
            eps_t = wgt.tile([P, 1], F32)
            nc.vector.memset(eps_t, EPS)
            ones_row = wgt.tile([1, P], F32)
            nc.vector.memset(ones_row, 1.0)
            if use_c0:
                c0row = wgt.tile([1, KH], F32)
                nc.sync.dma_start(out=c0row, in_=c0_d)

            x3_d = x_d.rearrange("(t p) c -> t p c", p=P)
            o3_d = out_d.rearrange("(t p) c -> t p c", p=P)

            # software pipeline: load two ahead, cast+stats one ahead
            x_ts = [None] * RT
            xr_ts = [None] * RT
            rstds = [None] * RT
            stds = [None] * RT

            def emit_load(t):
                x_t = big.tile([P, C], F32, tag="x", name=f"x_{t}", bufs=4)
                nc.sync.dma_start(out=x_t, in_=x3_d[t])
                x_ts[t] = x_t

            def emit_cast(t):
                # bf16 cast with the layernorm rstd folded in (DVE 2x single-src)
                xr_t = big.tile([P, C], BF16, tag="xr", name=f"xr_{t}", bufs=3)
                nc.vector.tensor_scalar(out=xr_t, in0=x_ts[t], scalar1=rstds[t],
                                        scalar2=None, op0=OP.mult)
                xr_ts[t] = xr_t

            def emit_stats2(t):
                # mean/var via bn_stats (DVE); rstd via sqrt(ACT) + reciprocal(DVE)
                bstats = sml.tile([P, NCOL, 6], F32, tag="bstats")
                xch = x_ts[t].rearrange("p (g f) -> p g f", f=512)
                for gi in range(NCOL):
                    nc.vector.bn_stats(out=bstats[:, gi], in_=xch[:, gi])
                mv = sml.tile([P, 2], F32, tag="mv")
                nc.vector.bn_aggr(out=mv, in_=bstats)
                std_t = sml.tile([P, 1], F32, tag="std", name=f"std_{t}")
                nc.scalar.activation(out=std_t, in_=mv[:, 1:2], func=AF.Sqrt,
                                     bias=eps_t, scale=1.0)
                rstd_t = sml.tile([P, 1], F32, tag="rstd", name=f"rstd_{t}")
                nc.vector.reciprocal(out=rstd_t, in_=std_t)
                rstds[t] = rstd_t
                stds[t] = std_t

            emit_load(0)
            emit_load(1)
            emit_load(2)
            emit_stats2(0)
            emit_stats2(1)
            emit_cast(0)
            emit_cast(1)

            for t in range(RT):
                x_t = x_ts[t]

                xr3 = xr_ts[t].rearrange("p (k d) -> p k d", d=NCOL)

                # prefetch next row tile (cast early so PE never waits on DVE)
                if t + 1 < RT:
                    emit_cast(t + 1)
                if t + 3 < RT:
                    emit_load(t + 3)

                # ---- transposes (PE) + Y matmuls, interleaved to keep PE dense
                planes = [None] * NCOL
                psums_y = [None] * NCOL
                psxs = pXS.tile([P, KH], F32, tag="pXS")
                v_t = big.tile([P, NCOL, KH], F32, tag="v")

                def emit_trans(d):
                    ps = pT.tile([P, KH], F32, tag="pT", name=f"pT_{t}_{d}")
                    for kc in range(KC):
                        nc.tensor.matmul(ps[:, kc * P:(kc + 1) * P],
                                         xr3[:, kc * P:(kc + 1) * P, d], identb)
                    pl = pln.tile([P, KH], BF16, tag="plane", name=f"pl_{t}_{d}")
                    if COPY_ENG[d] == "dve":
                        nc.vector.tensor_copy(out=pl, in_=ps)
                    else:
                        nc.scalar.copy(out=pl, in_=ps)
                    planes[d] = pl

                def emit_xs(kc):
                    for d in range(NCOL):
                        nc.tensor.matmul(psxs[:, kc * P:(kc + 1) * P],
                                         xr3[:, kc * P:(kc + 1) * P, d], identb,
                                         start=(d == 0), stop=(d == NCOL - 1))

                def emit_y(d):
                    acc = pY.tile([P, KH], F32, tag="pY", name=f"pY_{t}_{d}")
                    for kc in range(KC):
                        nc.tensor.matmul(acc, planes[d][:, kc * P:(kc + 1) * P],
                                         a2t[:, kc],
                                         start=(kc == 0),
                                         stop=(kc == KC - 1 and not use_c0))
                    psums_y[d] = acc

                def emit_tt(d):
                    nc.vector.tensor_tensor(out=v_t[:, d], in0=psums_y[d],
                                            in1=zsb, op=OP.add)

                # xsum first so Z (epilogue input) is ready early
                for kc in range(KC):
                    emit_xs(kc)
                xspl = med.tile([P, KH], BF16, tag="xsplane")
                nc.scalar.copy(out=xspl, in_=psxs)
                emit_trans(0)
                emit_trans(1)
                accz = pZ.tile([P, KH], F32, tag="pZ")
                for kc in range(KC):
                    nc.tensor.matmul(accz, xspl[:, kc * P:(kc + 1) * P], b3t[:, kc],
                                     start=(kc == 0), stop=(kc == KC - 1))
                zsb = med.tile([P, KH], F32, tag="zsb")
                nc.scalar.copy(out=zsb, in_=accz)

                emit_y(0)
                emit_tt(0)
                emit_trans(2)
                emit_y(1)
                emit_tt(1)
                emit_trans(3)
                emit_y(2)
                emit_tt(2)
                emit_trans(4)
                emit_y(3)
                emit_tt(3)
                emit_trans(5)
                emit_y(4)
                emit_tt(4)
                emit_y(5)
                emit_tt(5)
                if use_c0:
                    for d in range(NCOL):
                        nc.tensor.matmul(psums_y[d], ones_row, c0row,
                                         start=False, stop=True)

                # relu over the whole tile (ACT; scale already folded into planes)
                nc.scalar.activation(out=v_t, in_=v_t, func=AF.Relu)
                if t + 2 < RT:
                    emit_stats2(t + 2)
                # residual + re-interleave on GpSimd: out[p, k*6+d] = x + v[p, d, k]
                out_t = big.tile([P, C], F32, tag="out")
                nc.gpsimd.tensor_tensor(
                    out=out_t.rearrange("p (k d) -> p k d", d=NCOL),
                    in0=x_t.rearrange("p (k d) -> p k d", d=NCOL),
                    in1=v_t.rearrange("p d k -> p k d"),
                    op=OP.add)
                nc.scalar.dma_start(out=o3_d[t], in_=out_t)

    nc.compile()
    return nc


def _host_prep(conv_weight, conv_bias, norm_weight, norm_bias):
    w = conv_weight.astype(np.float64)
    g = norm_weight.astype(np.float64)
    bn = norm_bias.astype(np.float64)
    alpha = w[:, :, 0] - w[:, :, 1]
    betaw = w[:, :, 1]
    a2 = alpha * g[None, :]
    c1 = -(alpha @ g) - NCOL * (betaw @ g)
    b3 = betaw * g[None, :] + c1[:, None] / C
    c0 = alpha @ bn + NCOL * (betaw @ bn) + conv_bias.astype(np.float64)

    def chunk_kT(m):  # [o,k] -> [128, KC, o] with tile[p,kc,o] = m[o, kc*128+p]
        return np.ascontiguousarray(
            m.T.reshape(KC, P, KH).transpose(1, 0, 2).astype(np.float32))

    a2t = chunk_kT(a2)
    b3t = chunk_kT(b3)
    c0row = np.ascontiguousarray(c0.astype(np.float32).reshape(1, KH))
    use_c0 = bool(np.any(c0row != 0.0))
    ident = np.eye(P, dtype=np.float32)
    return a2t, b3t, c0row, use_c0, ident


def _run(inputs, trace=False):
    x = inputs["x"]
    a2t, b3t, c0row, use_c0, ident = _host_prep(
        inputs["conv_weight"], inputs["conv_bias"],
        inputs["norm_weight"], inputs["norm_bias"])
    key = use_c0
    if key not in _CACHE:
        _CACHE[key] = _build(use_c0)
    nc = _CACHE[key]

    shards = x.reshape(CORES, BL * NPOS, C).astype(np.float32, copy=False)
    in_maps = [{"x": np.ascontiguousarray(shards[i]), "a2t": a2t, "b3t": b3t,
                "ident": ident, "c0row": c0row} for i in range(CORES)]
    res = bass_utils.run_bass_kernel_spmd(nc, in_maps, list(range(CORES)),
                                          trace=trace)
    out = np.concatenate([r["out"].reshape(1, BL, NPOS, C) for r in res.results],
                         axis=0).reshape(B, NPOS, C)
    return out, res


def kernel(x, conv_weight, conv_bias, norm_weight, norm_bias):
    out, _ = _run({"x": x, "conv_weight": conv_weight, "conv_bias": conv_bias,
                   "norm_weight": norm_weight, "norm_bias": norm_bias})
    return out


# revision 19
# speedup vs baseline: 1.1803x; 1.1803x over previous
"""Trainium2 Bass kernel for nn_ColorResBlock (LayerNorm + color-structured conv + ReLU residual).

Math (reference):
    h  = LayerNorm(x) over last axis (3072), affine (gamma,beta) tied per k across 6 colors
    xr = h.reshape(B, 24, 512, 6)
    out[b,i,o,d] = sum_k alpha[o,k] xr[b,i,k,d] + sum_k beta[o,k] xsum[b,i,k] + bias[o]
    result = x + relu(out)

Key algebraic restructuring (all folds computed on host):
    A2[o,k]  = (W0-W1)[o,k] * gamma[k]
    C1[o]    = -(sum_k (W0-W1)[o,k] g[k]) - 6*sum_k W1[o,k] g[k]
    B3[o,k]  = W1[o,k]*g[k] + C1[o]/3072          (folds the mean*C1 rank-1 term)
    C0[o]    = (W0-W1)@bn + 6*(W1@bn) + bias
    h_d[r,o] = rstd[r] * ( A2 @ x_d[r] + B3 @ xsum[r] + std[r]*C0[o] )
so the matmuls run on RAW x (de-interleaved+transposed on device via PE), and the
per-row layernorm enters only as a per-partition scale folded into the ReLU pass.

Data parallel over batch: 8 cores x 128 batches.  Per core: rows R=3072, channels 3072.
"""

import numpy as np
import concourse.bacc as bacc
import concourse.bass as bass
import concourse.tile as tile
from concourse import mybir, bass_utils

F32 = mybir.dt.float32
F32R = mybir.dt.float32r
BF16 = mybir.dt.bfloat16
AF = mybir.ActivationFunctionType
OP = mybir.AluOpType

B, NPOS, KH, NCOL = 1024, 24, 512, 6
CORES = 8
BL = B // CORES            # batches per core
R = BL * NPOS              # 3072 rows per core
C = KH * NCOL              # 3072 channels
P = 128                    # partitions per row tile
RT = R // P                # 24 row tiles
KC = KH // P               # 4 contraction chunks of 128
EPS = 1e-5

# d-plane -> engine for the PSUM->SBUF transpose copies ("dve" or "act")
COPY_ENG = ["act", "act", "act", "act", "act", "act"]

_CACHE = {}


def _build(use_c0: bool):
    nc = bacc.Bacc("TRN2", target_bir_lowering=False, debug=False)
    x_d = nc.dram_tensor("x", [R, C], F32, kind="ExternalInput").ap()
    a2_d = nc.dram_tensor("a2t", [P, KC, KH], F32, kind="ExternalInput").ap()
    b3_d = nc.dram_tensor("b3t", [P, KC, KH], F32, kind="ExternalInput").ap()
    id_d = nc.dram_tensor("ident", [P, P], F32, kind="ExternalInput").ap()
    c0_d = nc.dram_tensor("c0row", [1, KH], F32, kind="ExternalInput").ap()
    out_d = nc.dram_tensor("out", [R, C], F32, kind="ExternalOutput").ap()

    with tile.TileContext(nc) as tc:
        with tc.tile_pool(name="wgt", bufs=1) as wgt, \
             tc.tile_pool(name="big", bufs=2) as big, \
             tc.tile_pool(name="med", bufs=2) as med, \
             tc.tile_pool(name="pln", bufs=6) as pln, \
             tc.tile_pool(name="sml", bufs=2) as sml, \
             tc.tile_pool(name="pT", bufs=3, space="PSUM") as pT, \
             tc.tile_pool(name="pXS", bufs=1, space="PSUM") as pXS, \
             tc.tile_pool(name="pY", bufs=(2 if use_c0 else 3), space="PSUM") as pY, \
             tc.tile_pool(name="pZ", bufs=1, space="PSUM") as pZ:

            # ---- one-time setup: weights + identity rounded to f32r
            stage = wgt.tile([P, KC, KH], F32, tag="stage")
            nc.sync.dma_start(out=stage, in_=a2_d)
            a2t = wgt.tile([P, KC, KH], BF16)
            nc.vector.tensor_copy(out=a2t, in_=stage)
            stage2 = wgt.tile([P, KC, KH], F32, tag="stage")
            nc.sync.dma_start(out=stage2, in_=b3_d)
            b3t = wgt.tile([P, KC, KH], BF16)
            nc.vector.tensor_copy(out=b3t, in_=stage2)
            idstage = wgt.tile([P, P], F32, tag="idstage")
            nc.sync.dma_start(out=idstage, in_=id_d)
            identb = wgt.tile([P, P], BF16)
            nc.vector.tensor_copy(out=identb, in_=idstage)
            identf = idstage
            eps_t = wgt.tile([P, 1], F32)
            nc.vector.memset(eps_t, EPS)
            ones_row = wgt.tile([1, P], F32)
            nc.vector.memset(ones_row, 1.0)
            if use_c0:
                c0row = wgt.tile([1, KH], F32)
                nc.sync.dma_start(out=c0row, in_=c0_d)

            x3_d = x_d.rearrange("(t p) c -> t p c", p=P)
            o3_d = out_d.rearrange("(t p) c -> t p c", p=P)

            # software pipeline: load two ahead, cast+stats one ahead
            x_ts = [None] * RT
            xr_ts = [None] * RT
            rstds = [None] * RT
            stds = [None] * RT

            def emit_load(t):
                x_t = big.tile([P, C], F32, tag="x", name=f"x_{t}", bufs=4)
                nc.sync.dma_start(out=x_t, in_=x3_d[t])
                x_ts[t] = x_t

            def emit_cast(t):
                # bf16 cast with the layernorm rstd folded in (DVE 2x single-src),
                # chunked so the first transposes of the next tile start early
                xr_t = big.tile([P, C], BF16, tag="xr", name=f"xr_{t}")
                q = C // KC
                for kc in range(KC):
                    nc.vector.tensor_scalar(out=xr_t[:, kc * q:(kc + 1) * q],
                                            in0=x_ts[t][:, kc * q:(kc + 1) * q],
                                            scalar1=rstds[t],
                                            scalar2=None, op0=OP.mult)
                xr_ts[t] = xr_t

            def emit_stats2(t):
                # mean/var via bn_stats (DVE); rstd via sqrt(ACT) + reciprocal(DVE)
                bstats = sml.tile([P, NCOL, 6], F32, tag="bstats")
                xch = x_ts[t].rearrange("p (g f) -> p g f", f=512)
                for gi in range(NCOL):
                    nc.vector.bn_stats(out=bstats[:, gi], in_=xch[:, gi])
                mv = sml.tile([P, 2], F32, tag="mv")
                nc.vector.bn_aggr(out=mv, in_=bstats)
                std_t = sml.tile([P, 1], F32, tag="std", name=f"std_{t}")
                nc.scalar.activation(out=std_t, in_=mv[:, 1:2], func=AF.Sqrt,
                                     bias=eps_t, scale=1.0)
                rstd_t = sml.tile([P, 1], F32, tag="rstd", name=f"rstd_{t}")
                nc.vector.reciprocal(out=rstd_t, in_=std_t)
                rstds[t] = rstd_t
                stds[t] = std_t

            emit_load(0)
            emit_load(1)
            emit_load(2)
            emit_stats2(0)
            emit_stats2(1)
            emit_cast(0)
            emit_cast(1)

            for t in range(RT):
                x_t = x_ts[t]

                xr3 = xr_ts[t].rearrange("p (k d) -> p k d", d=NCOL)

                # prefetch next row tile (cast early so PE never waits on DVE)
                if t + 1 < RT:
                    emit_cast(t + 1)
                if t + 3 < RT:
                    emit_load(t + 3)

                # ---- transposes (PE) + Y matmuls, interleaved to keep PE dense
                planes = [None] * NCOL
                psums_y = [None] * NCOL
                psxs = pXS.tile([P, KH], F32, tag="pXS")
                v_t = big.tile([P, NCOL, KH], F32, tag="v")

                def emit_trans(d):
                    ps = pT.tile([P, KH], F32, tag="pT", name=f"pT_{t}_{d}")
                    for kc in range(KC):
                        nc.tensor.matmul(ps[:, kc * P:(kc + 1) * P],
                                         xr3[:, kc * P:(kc + 1) * P, d], identb)
                    pl = pln.tile([P, KH], BF16, tag="plane", name=f"pl_{t}_{d}")
                    if COPY_ENG[d] == "dve":
                        nc.vector.tensor_copy(out=pl, in_=ps)
                    else:
                        nc.scalar.copy(out=pl, in_=ps)
                    planes[d] = pl

                def emit_xs(kc):
                    for d in range(NCOL):
                        nc.tensor.matmul(psxs[:, kc * P:(kc + 1) * P],
                                         xr3[:, kc * P:(kc + 1) * P, d], identb,
                                         start=(d == 0), stop=(d == NCOL - 1))

                def emit_y(d):
                    acc = pY.tile([P, KH], F32, tag="pY", name=f"pY_{t}_{d}")
                    for kc in range(KC):
                        nc.tensor.matmul(acc, planes[d][:, kc * P:(kc + 1) * P],
                                         a2t[:, kc],
                                         start=(kc == 0),
                                         stop=(kc == KC - 1 and not use_c0))
                    psums_y[d] = acc

                def emit_tt(d):
                    nc.vector.tensor_tensor(out=v_t[:, d], in0=psums_y[d],
                                            in1=zsb, op=OP.add)

                # xsum first so Z (epilogue input) is ready early
                for kc in range(KC):
                    emit_xs(kc)
                xspl = med.tile([P, KH], BF16, tag="xsplane")
                nc.scalar.copy(out=xspl, in_=psxs)
                emit_trans(0)
                emit_trans(1)
                accz = pZ.tile([P, KH], F32, tag="pZ")
                for kc in range(KC):
                    nc.tensor.matmul(accz, xspl[:, kc * P:(kc + 1) * P], b3t[:, kc],
                                     start=(kc == 0), stop=(kc == KC - 1))
                zsb = med.tile([P, KH], F32, tag="zsb")
                nc.scalar.copy(out=zsb, in_=accz)

                emit_trans(2)
                emit_trans(3)
                emit_trans(4)
                emit_trans(5)
                emit_y(0)
                emit_tt(0)
                emit_y(1)
                emit_tt(1)
                emit_y(2)
                emit_tt(2)
                emit_y(3)
                emit_tt(3)
                emit_y(4)
                emit_tt(4)
                emit_y(5)
                emit_tt(5)
                if use_c0:
                    for d in range(NCOL):
                        nc.tensor.matmul(psums_y[d], ones_row, c0row,
                                         start=False, stop=True)

                # relu over the whole tile (ACT; scale already folded into planes)
                nc.scalar.activation(out=v_t, in_=v_t, func=AF.Relu)
                if t + 2 < RT:
                    emit_stats2(t + 2)
                # residual + re-interleave on GpSimd: out[p, k*6+d] = x + v[p, d, k]
                out_t = big.tile([P, C], F32, tag="out")
                nc.gpsimd.tensor_tensor(
                    out=out_t.rearrange("p (k d) -> p k d", d=NCOL),
                    in0=x_t.rearrange("p (k d) -> p k d", d=NCOL),
                    in1=v_t.rearrange("p d k -> p k d"),
                    op=OP.add)
                nc.scalar.dma_start(out=o3_d[t], in_=out_t)

    nc.compile()
    return nc


def _host_prep(conv_weight, conv_bias, norm_weight, norm_bias):
    w = conv_weight.astype(np.float64)
    g = norm_weight.astype(np.float64)
    bn = norm_bias.astype(np.float64)
    alpha = w[:, :, 0] - w[:, :, 1]
    betaw = w[:, :, 1]
    a2 = alpha * g[None, :]
    c1 = -(alpha @ g) - NCOL * (betaw @ g)
    b3 = betaw * g[None, :] + c1[:, None] / C
    c0 = alpha @ bn + NCOL * (betaw @ bn) + conv_bias.astype(np.float64)

    def chunk_kT(m):  # [o,k] -> [128, KC, o] with tile[p,kc,o] = m[o, kc*128+p]
        return np.ascontiguousarray(
            m.T.reshape(KC, P, KH).transpose(1, 0, 2).astype(np.float32))

    a2t = chunk_kT(a2)
    b3t = chunk_kT(b3)
    c0row = np.ascontiguousarray(c0.astype(np.float32).reshape(1, KH))
    use_c0 = bool(np.any(c0row != 0.0))
    ident = np.eye(P, dtype=np.float32)
    return a2t, b3t, c0row, use_c0, ident


def _run(inputs, trace=False):
    x = inputs["x"]
    a2t, b3t, c0row, use_c0, ident = _host_prep(
        inputs["conv_weight"], inputs["conv_bias"],
        inputs["norm_weight"], inputs["norm_bias"])
    key = use_c0
    if key not in _CACHE:
        _CACHE[key] = _build(use_c0)
    nc = _CACHE[key]

    shards = x.reshape(CORES, BL * NPOS, C).astype(np.float32, copy=False)
    in_maps = [{"x": np.ascontiguousarray(shards[i]), "a2t": a2t, "b3t": b3t,
                "ident": ident, "c0row": c0row} for i in range(CORES)]
    res = bass_utils.run_bass_kernel_spmd(nc, in_maps, list(range(CORES)),
                                          trace=trace)
    out = np.concatenate([r["out"].reshape(1, BL, NPOS, C) for r in res.results],
                         axis=0).reshape(B, NPOS, C)
    return out, res


def kernel(x, conv_weight, conv_bias, norm_weight, norm_bias):
    out, _ = _run({"x": x, "conv_weight": conv_weight, "conv_bias": conv_bias,
                   "norm_weight": norm_weight, "norm_bias": norm_bias})
    return out


# revision 20
# speedup vs baseline: 1.1876x; 1.0062x over previous
"""Trainium2 Bass kernel for nn_ColorResBlock (LayerNorm + color-structured conv + ReLU residual).

Math (reference):
    h  = LayerNorm(x) over last axis (3072), affine (gamma,beta) tied per k across 6 colors
    xr = h.reshape(B, 24, 512, 6)
    out[b,i,o,d] = sum_k alpha[o,k] xr[b,i,k,d] + sum_k beta[o,k] xsum[b,i,k] + bias[o]
    result = x + relu(out)

Key algebraic restructuring (all folds computed on host):
    A2[o,k]  = (W0-W1)[o,k] * gamma[k]
    C1[o]    = -(sum_k (W0-W1)[o,k] g[k]) - 6*sum_k W1[o,k] g[k]
    B3[o,k]  = W1[o,k]*g[k] + C1[o]/3072          (folds the mean*C1 rank-1 term)
    C0[o]    = (W0-W1)@bn + 6*(W1@bn) + bias
    h_d[r,o] = rstd[r] * ( A2 @ x_d[r] + B3 @ xsum[r] + std[r]*C0[o] )
so the matmuls run on RAW x (de-interleaved+transposed on device via PE), and the
per-row layernorm enters only as a per-partition scale folded into the ReLU pass.

Data parallel over batch: 8 cores x 128 batches.  Per core: rows R=3072, channels 3072.
"""

import numpy as np
import concourse.bacc as bacc
import concourse.bass as bass
import concourse.tile as tile
from concourse import mybir, bass_utils

F32 = mybir.dt.float32
F32R = mybir.dt.float32r
BF16 = mybir.dt.bfloat16
AF = mybir.ActivationFunctionType
OP = mybir.AluOpType

B, NPOS, KH, NCOL = 1024, 24, 512, 6
CORES = 8
BL = B // CORES            # batches per core
R = BL * NPOS              # 3072 rows per core
C = KH * NCOL              # 3072 channels
P = 128                    # partitions per row tile
RT = R // P                # 24 row tiles
KC = KH // P               # 4 contraction chunks of 128
EPS = 1e-5

# d-plane -> engine for the PSUM->SBUF transpose copies ("dve" or "act")
COPY_ENG = ["act", "act", "act", "act", "act", "act"]

_CACHE = {}


def _build(use_c0: bool):
    nc = bacc.Bacc("TRN2", target_bir_lowering=False, debug=False)
    x_d = nc.dram_tensor("x", [R, C], F32, kind="ExternalInput").ap()
    a2_d = nc.dram_tensor("a2t", [P, KC, KH], F32, kind="ExternalInput").ap()
    b3_d = nc.dram_tensor("b3t", [P, KC, KH], F32, kind="ExternalInput").ap()
    id_d = nc.dram_tensor("ident", [P, P], F32, kind="ExternalInput").ap()
    c0_d = nc.dram_tensor("c0row", [1, KH], F32, kind="ExternalInput").ap()
    out_d = nc.dram_tensor("out", [R, C], F32, kind="ExternalOutput").ap()

    with tile.TileContext(nc) as tc:
        with tc.tile_pool(name="wgt", bufs=1) as wgt, \
             tc.tile_pool(name="big", bufs=2) as big, \
             tc.tile_pool(name="med", bufs=2) as med, \
             tc.tile_pool(name="pln", bufs=6) as pln, \
             tc.tile_pool(name="sml", bufs=2) as sml, \
             tc.tile_pool(name="pT", bufs=3, space="PSUM") as pT, \
             tc.tile_pool(name="pXS", bufs=1, space="PSUM") as pXS, \
             tc.tile_pool(name="pY", bufs=(2 if use_c0 else 3), space="PSUM") as pY, \
             tc.tile_pool(name="pZ", bufs=1, space="PSUM") as pZ:

            # ---- one-time setup: weights + identity rounded to f32r
            stage = wgt.tile([P, KC, KH], F32, tag="stage")
            nc.sync.dma_start(out=stage, in_=a2_d)
            a2t = wgt.tile([P, KC, KH], BF16)
            nc.vector.tensor_copy(out=a2t, in_=stage)
            stage2 = wgt.tile([P, KC, KH], F32, tag="stage")
            nc.sync.dma_start(out=stage2, in_=b3_d)
            b3t = wgt.tile([P, KC, KH], BF16)
            nc.vector.tensor_copy(out=b3t, in_=stage2)
            idstage = wgt.tile([P, P], F32, tag="idstage")
            nc.sync.dma_start(out=idstage, in_=id_d)
            identb = wgt.tile([P, P], BF16)
            nc.vector.tensor_copy(out=identb, in_=idstage)
            identf = idstage
            eps_t = wgt.tile([P, 1], F32)
            nc.vector.memset(eps_t, EPS)
            ones_row = wgt.tile([1, P], F32)
            nc.vector.memset(ones_row, 1.0)
            if use_c0:
                c0row = wgt.tile([1, KH], F32)
                nc.sync.dma_start(out=c0row, in_=c0_d)

            x3_d = x_d.rearrange("(t p) c -> t p c", p=P)
            o3_d = out_d.rearrange("(t p) c -> t p c", p=P)

            # software pipeline: load two ahead, cast+stats one ahead
            x_ts = [None] * RT
            xr_ts = [None] * RT
            rstds = [None] * RT
            stds = [None] * RT

            def emit_load(t):
                x_t = big.tile([P, C], F32, tag="x", name=f"x_{t}", bufs=4)
                nc.sync.dma_start(out=x_t, in_=x3_d[t])
                x_ts[t] = x_t

            def emit_cast(t):
                # bf16 cast with the layernorm rstd folded in (DVE 2x single-src),
                # chunked so the first transposes of the next tile start early
                xr_t = big.tile([P, C], BF16, tag="xr", name=f"xr_{t}")
                q = C // KC
                for kc in range(KC):
                    nc.vector.tensor_scalar(out=xr_t[:, kc * q:(kc + 1) * q],
                                            in0=x_ts[t][:, kc * q:(kc + 1) * q],
                                            scalar1=rstds[t],
                                            scalar2=None, op0=OP.mult)
                xr_ts[t] = xr_t

            def emit_stats2(t):
                # mean/var via bn_stats (DVE); rstd via sqrt(ACT) + reciprocal(DVE)
                bstats = sml.tile([P, NCOL, 6], F32, tag="bstats")
                xch = x_ts[t].rearrange("p (g f) -> p g f", f=512)
                for gi in range(NCOL):
                    nc.vector.bn_stats(out=bstats[:, gi], in_=xch[:, gi])
                mv = sml.tile([P, 2], F32, tag="mv")
                nc.vector.bn_aggr(out=mv, in_=bstats)
                std_t = sml.tile([P, 1], F32, tag="std", name=f"std_{t}")
                nc.scalar.activation(out=std_t, in_=mv[:, 1:2], func=AF.Sqrt,
                                     bias=eps_t, scale=1.0)
                rstd_t = sml.tile([P, 1], F32, tag="rstd", name=f"rstd_{t}")
                nc.vector.reciprocal(out=rstd_t, in_=std_t)
                rstds[t] = rstd_t
                stds[t] = std_t

            emit_load(0)
            emit_load(1)
            emit_load(2)
            emit_stats2(0)
            emit_stats2(1)
            emit_cast(0)
            emit_cast(1)

            for t in range(RT):
                x_t = x_ts[t]

                xr3 = xr_ts[t].rearrange("p (k d) -> p k d", d=NCOL)

                # prefetch next row tile (cast early so PE never waits on DVE)
                if t + 1 < RT:
                    emit_cast(t + 1)
                if t + 3 < RT:
                    emit_load(t + 3)

                # ---- transposes (PE) + Y matmuls, interleaved to keep PE dense
                planes = [None] * NCOL
                psums_y = [None] * NCOL
                psxs = pXS.tile([P, KH], F32, tag="pXS")
                v_t = big.tile([P, NCOL, KH], F32, tag="v", bufs=3)

                def emit_trans(d):
                    ps = pT.tile([P, KH], F32, tag="pT", name=f"pT_{t}_{d}")
                    for kc in range(KC):
                        nc.tensor.matmul(ps[:, kc * P:(kc + 1) * P],
                                         xr3[:, kc * P:(kc + 1) * P, d], identb)
                    pl = pln.tile([P, KH], BF16, tag="plane", name=f"pl_{t}_{d}")
                    if COPY_ENG[d] == "dve":
                        nc.vector.tensor_copy(out=pl, in_=ps)
                    else:
                        nc.scalar.copy(out=pl, in_=ps)
                    planes[d] = pl

                def emit_xs(kc):
                    for d in range(NCOL):
                        nc.tensor.matmul(psxs[:, kc * P:(kc + 1) * P],
                                         xr3[:, kc * P:(kc + 1) * P, d], identb,
                                         start=(d == 0), stop=(d == NCOL - 1))

                def emit_y(d):
                    acc = pY.tile([P, KH], F32, tag="pY", name=f"pY_{t}_{d}")
                    for kc in range(KC):
                        nc.tensor.matmul(acc, planes[d][:, kc * P:(kc + 1) * P],
                                         a2t[:, kc],
                                         start=(kc == 0),
                                         stop=(kc == KC - 1 and not use_c0))
                    psums_y[d] = acc

                def emit_tt(d):
                    nc.vector.tensor_tensor(out=v_t[:, d], in0=psums_y[d],
                                            in1=zsb, op=OP.add)

                # xsum first so Z (epilogue input) is ready early
                for kc in range(KC):
                    emit_xs(kc)
                xspl = med.tile([P, KH], BF16, tag="xsplane")
                nc.scalar.copy(out=xspl, in_=psxs)
                emit_trans(0)
                emit_trans(1)
                accz = pZ.tile([P, KH], F32, tag="pZ")
                for kc in range(KC):
                    nc.tensor.matmul(accz, xspl[:, kc * P:(kc + 1) * P], b3t[:, kc],
                                     start=(kc == 0), stop=(kc == KC - 1))
                zsb = med.tile([P, KH], F32, tag="zsb")
                nc.scalar.copy(out=zsb, in_=accz)

                emit_trans(2)
                emit_trans(3)
                emit_trans(4)
                emit_trans(5)
                emit_y(0)
                emit_tt(0)
                emit_y(1)
                emit_tt(1)
                emit_y(2)
                emit_tt(2)
                emit_y(3)
                emit_tt(3)
                emit_y(4)
                emit_tt(4)
                emit_y(5)
                emit_tt(5)
                if use_c0:
                    for d in range(NCOL):
                        nc.tensor.matmul(psums_y[d], ones_row, c0row,
                                         start=False, stop=True)

                # relu over the whole tile (ACT; scale already folded into planes)
                nc.scalar.activation(out=v_t, in_=v_t, func=AF.Relu)
                if t + 2 < RT:
                    emit_stats2(t + 2)
                # residual + re-interleave on GpSimd: out[p, k*6+d] = x + v[p, d, k]
                out_t = big.tile([P, C], F32, tag="out")
                nc.gpsimd.tensor_tensor(
                    out=out_t.rearrange("p (k d) -> p k d", d=NCOL),
                    in0=x_t.rearrange("p (k d) -> p k d", d=NCOL),
                    in1=v_t.rearrange("p d k -> p k d"),
                    op=OP.add)
                nc.scalar.dma_start(out=o3_d[t], in_=out_t)

    nc.compile()
    return nc


def _host_prep(conv_weight, conv_bias, norm_weight, norm_bias):
    w = conv_weight.astype(np.float64)
    g = norm_weight.astype(np.float64)
    bn = norm_bias.astype(np.float64)
    alpha = w[:, :, 0] - w[:, :, 1]
    betaw = w[:, :, 1]
    a2 = alpha * g[None, :]
    c1 = -(alpha @ g) - NCOL * (betaw @ g)
    b3 = betaw * g[None, :] + c1[:, None] / C
    c0 = alpha @ bn + NCOL * (betaw @ bn) + conv_bias.astype(np.float64)

    def chunk_kT(m):  # [o,k] -> [128, KC, o] with tile[p,kc,o] = m[o, kc*128+p]
        return np.ascontiguousarray(
            m.T.reshape(KC, P, KH).transpose(1, 0, 2).astype(np.float32))

    a2t = chunk_kT(a2)
    b3t = chunk_kT(b3)
    c0row = np.ascontiguousarray(c0.astype(np.float32).reshape(1, KH))
    use_c0 = bool(np.any(c0row != 0.0))
    ident = np.eye(P, dtype=np.float32)
    return a2t, b3t, c0row, use_c0, ident


def _run(inputs, trace=False):
    x = inputs["x"]
    a2t, b3t, c0row, use_c0, ident = _host_prep(
        inputs["conv_weight"], inputs["conv_bias"],
        inputs["norm_weight"], inputs["norm_bias"])
    key = use_c0
    if key not in _CACHE:
        _CACHE[key] = _build(use_c0)
    nc = _CACHE[key]

    shards = x.reshape(CORES, BL * NPOS, C).astype(np.float32, copy=False)
    in_maps = [{"x": np.ascontiguousarray(shards[i]), "a2t": a2t, "b3t": b3t,
                "ident": ident, "c0row": c0row} for i in range(CORES)]
    res = bass_utils.run_bass_kernel_spmd(nc, in_maps, list(range(CORES)),
                                          trace=trace)
    out = np.concatenate([r["out"].reshape(1, BL, NPOS, C) for r in res.results],
                         axis=0).reshape(B, NPOS, C)
    return out, res


def kernel(x, conv_weight, conv_bias, norm_weight, norm_bias):
    out, _ = _run({"x": x, "conv_weight": conv_weight, "conv_bias": conv_bias,
                   "norm_weight": norm_weight, "norm_bias": norm_bias})
    return out
